# revision 15
# baseline (speedup 1.0000x reference)
"""Trainium2 Bass kernel for nn_MultiHeadAttention_39324720562623.

Reference computation (N=4, T=2048, D=512, H=8, HD=64), fp32:
    keys   = query @ Wk.T + query
    values = query @ Wv.T
    per head h: scores = softmax((Q_h @ K_h.T) / sqrt(HD))
                out_h  = scores @ V_h
    out = concat_heads(out_h) @ Wo.T

Sharding: 8 cores = 4 batches x 2 head-groups (4 heads each), pure SPMD.
Each core computes K/V projections for its head-group only, attention for
its 4 heads, and a partial O-projection; the host sums the two partials
per batch.  All operands are pre-transposed on the host so the kernel
works in "feature-major" (transposed) layouts throughout:

  qT   [512,2048]  query[n].T with feature dims permuted so this core's
                   head-group dims come first
  wkiT [512, 256]  ((Wk + I)[hg,:][:,perm]).T  -> K.T = wkiT.T @ qT
                   (residual folded into the weight)
  wvT  [512, 256]  (Wv[hg,:][:,perm]).T        -> V = qT_chunk.T @ wvT
  woT  [256, 512]  as [64,4,512]-striped: Wo.T rows for this head-group
  outT [512,2048]  partial out.T (host sums pairs, then transposes)

Softmax needs no max-subtraction: scores/8 are bounded (|s|<~16) so fp32
exp is safe.  The denominator comes for free from a ones-column appended
to V (row 64 of the att.T accumulation); normalization broadcasts the
reciprocal across partitions with a 1x64 ones outer-product on the PE.

All matmuls run as float32r (full PE rate at free-dim >= 256).
"""

import os
import sys

for _p in ("/opt/trn_rl_repo", os.path.join(os.path.expanduser("~"), ".axon_site", "_ro", "trn_rl_repo")):
    if os.path.isdir(_p) and _p not in sys.path:
        sys.path.insert(0, _p)
        break

import numpy as np

import concourse.bass as bass
import concourse.tile as tile
from concourse import mybir
from concourse.bass_utils import run_bass_kernel_spmd
from concourse.vector_clock import ScopedClock

N, T, D, H = 4, 2048, 512, 8
HD = D // H            # 64 head dim
HG = 2                 # head groups (cores per batch)
HPG = H // HG          # 4 heads per group
DG = D // HG           # 256 feature dims per group
F32 = mybir.dt.float32
F32R = mybir.dt.float32r
EXP = mybir.ActivationFunctionType.Exp
MULT = mybir.AluOpType.mult

QT = 512               # q-tile (matmul moving free dim)
NQT = T // QT          # 4
KC = 128               # k-chunk (partition dim of scores.T tiles)
NKC = T // KC          # 16
SGRP = 2               # k-chunks per scores-psum tile / exp instruction
NJUNK = 1              # PE-warmth filler matmuls per group (keeps HAM at K=8/8)
SCW = SGRP * QT        # 1024 scores tile free width


def _patch_drain():
    """walrus in this toolchain rejects >1 sync-wait on the kernel-tail
    Drain; split the waits across a chain of drains (1 wait each)."""
    if getattr(tile.TileContext, "_drain_split_patch", False):
        return

    def _drain_and_barrier(self, tick_clock, wait_clock):
        nc = self.nc
        d = nc.sync.drain()
        wait_clock.add_sem_waits(d.ins, ScopedClock({None: tick_clock.global_clock}))
        si = d.ins.sync_info
        waits = list(si.on_wait) if (si is not None and si.on_wait) else []
        if len(waits) > 1:
            si.on_wait = waits[:1]
            for w in waits[1:]:
                d2 = nc.sync.drain()
                if d2.ins.sync_info is None:
                    d2.ins.sync_info = mybir.SyncInfo(on_wait=[w], on_update=[])
                else:
                    d2.ins.sync_info.on_wait = [w]
        nc.all_engine_barrier()
        popped = nc._tile_sem_poison_stack.pop()
        assert popped is self._sem_poison
        nc.clear_and_free_semaphores(list(self.sems.allocated().values()))
        nc.all_engine_barrier()

    tile.TileContext._drain_and_barrier = _drain_and_barrier
    tile.TileContext._drain_split_patch = True


MAX_WAITS = 1


def _split_excess_waits(nc, maxw=MAX_WAITS):
    """walrus codegen rejects instructions with more than ~2 sync waits.
    Hoist excess waits onto same-engine nops inserted immediately before
    the offending instruction (same engine-stream position => identical
    semantics)."""
    nid = [0]

    def mk_nop(engine, waits):
        nid[0] += 1
        nop = mybir.InstNoOp(name=f"I-waitsplit-{nid[0]}")
        nop.engine = engine
        nop.sync_info = mybir.SyncInfo(on_wait=list(waits), on_update=[])
        try:
            nop.bass_nofuse = True
        except Exception:
            pass
        return nop

    for f in nc.m.functions:
        for bb in f.blocks:
            insts = bb.instructions
            i = 0
            while i < len(insts):
                ins = insts[i]
                si = ins.sync_info
                waits = list(si.on_wait) if (si is not None and si.on_wait) else []
                if len(waits) > maxw:
                    si.on_wait = waits[-maxw:]
                    excess = waits[:-maxw]
                    pos = i
                    for j in range(0, len(excess), maxw):
                        insts.insert(pos, mk_nop(ins.engine, excess[j : j + maxw]))
                        pos += 1
                        i += 1
                i += 1


def build_program():
    _patch_drain()
    nc = bass.Bass()
    qT = nc.dram_tensor("qT", [D, T], F32R, kind="ExternalInput")
    wkiT = nc.dram_tensor("wkiT", [D, DG], F32R, kind="ExternalInput")
    wvT = nc.dram_tensor("wvT", [D, DG], F32R, kind="ExternalInput")
    woT = nc.dram_tensor("woT", [DG, D], F32R, kind="ExternalInput")
    ones = nc.dram_tensor("ones", [128, 512], F32R, kind="ExternalInput")
    outT = nc.dram_tensor("outT", [D, T], F32, kind="ExternalOutput")

    with tile.TileContext(nc) as tc:
        with (
            tc.tile_pool(name="singles", bufs=1) as singles,
            tc.tile_pool(name="pt", bufs=3) as ptp,
            tc.tile_pool(name="recp", bufs=3) as recp,
            tc.tile_pool(name="outp", bufs=3) as outp,
            tc.tile_pool(name="ps_sc", bufs=2, space="PSUM") as ps_sc,
            tc.tile_pool(name="ps_att", bufs=1, space="PSUM") as ps_att,
            tc.tile_pool(name="ps_warm", bufs=1, space="PSUM") as ps_warm,
            tc.tile_pool(name="ps_aux", bufs=2, space="PSUM") as ps_aux,
        ):
            qT_sb = singles.tile([128, 4, T], F32R)
            wkiT_sb = singles.tile([128, 4, DG], F32R)
            wvT_sb = singles.tile([128, 4, DG], F32R)
            woT_sb = singles.tile([64, HPG, D], F32R)
            kT_sb = singles.tile([128, 2, T], F32R)
            # V augmented with a ones column per head: [128, kchunk, head, 65]
            vaug_sb = singles.tile([128, NKC, HPG, HD + 1], F32R)
            attT_sb = singles.tile([64, HPG, T], F32R)
            ones_sb = singles.tile([65, 512], F32R)

            # ---- input DMAs (chunked so compute can start early) ----
            for c in range(4):
                nc.sync.dma_start(
                    qT_sb[:, c], qT.rearrange("(c p) t -> c p t", p=128)[c]
                )
            nc.sync.dma_start(wkiT_sb[:], wkiT.rearrange("(c p) d -> p c d", p=128))
            nc.sync.dma_start(wvT_sb[:], wvT.rearrange("(c p) d -> p c d", p=128))
            nc.sync.dma_start(woT_sb[:], woT.rearrange("(c p) d -> p c d", p=64))
            nc.sync.dma_start(ones_sb[:], ones[0:65, :])
            nc.sync.dma_start(
                vaug_sb[:, :, :, HD],
                ones[:, 0:64].rearrange("p (a b) -> p a b", b=HPG),
            )

            # ---- K.T = (Wk+I) @ q.T for this head-group: [256, 2048] ----
            for dc in range(2):
                for tt in range(NQT):
                    ps = ps_aux.tile([128, QT], F32, tag="aux")
                    for di in range(4):
                        nc.tensor.matmul(
                            ps[:],
                            wkiT_sb[:, di, 128 * dc : 128 * (dc + 1)],
                            qT_sb[:, di, QT * tt : QT * (tt + 1)],
                            start=(di == 0),
                            stop=(di == 3),
                        )
                    nc.vector.tensor_copy(kT_sb[:, dc, QT * tt : QT * (tt + 1)], ps[:])

            # ---- V = q @ Wv.T for this head-group: [2048, 256] ----
            for tci in range(NKC):
                ps = ps_aux.tile([128, DG], F32, tag="aux")
                for di in range(4):
                    nc.tensor.matmul(
                        ps[:],
                        qT_sb[:, di, 128 * tci : 128 * (tci + 1)],
                        wvT_sb[:, di],
                        start=(di == 0),
                        stop=(di == 3),
                    )
                nc.vector.tensor_copy(
                    vaug_sb[:, tci, :, 0:HD],
                    ps.rearrange("p (h d) -> p h d", d=HD),
                )

            # ---- attention per (q-tile, head) ----
            for jq in range(NQT):
                qsl = slice(QT * jq, QT * (jq + 1))
                for h in range(HPG):
                    ch, po = h // 2, (h % 2) * 64
                    att_ps = ps_att.tile([HD + 1, QT], F32, tag="att")
                    nmm = 0
                    for g0 in range(0, NKC, SGRP):
                        sc_ps = ps_sc.tile([128, SCW], F32, tag="sc")
                        for j in range(SGRP):
                            ik = g0 + j
                            nc.tensor.matmul(
                                sc_ps[:, QT * j : QT * (j + 1)],
                                kT_sb[po : po + 64, ch, 128 * ik : 128 * (ik + 1)],
                                qT_sb[po : po + 64, ch, qsl],
                                start=True,
                                stop=True,
                            )
                        pt = ptp.tile([128, SCW], F32R, tag="pt")
                        nc.scalar.activation(pt[:], sc_ps[:], EXP, scale=0.125)
                        for j in range(SGRP):
                            ik = g0 + j
                            nc.tensor.matmul(
                                att_ps[:],
                                vaug_sb[:, ik, h],
                                pt[:, QT * j : QT * (j + 1)],
                                start=(nmm == 0),
                                stop=(nmm == NKC - 1),
                            )
                            nmm += 1
                        for _ in range(NJUNK):
                            wps = ps_warm.tile([64, QT], F32, tag="warm")
                            nc.tensor.matmul(
                                wps[0:1, :],
                                ones_sb[0:1, 0:1],
                                ones_sb[0:1, :],
                                start=True,
                                stop=True,
                            )
                    # normalization: recip of the ones-row, broadcast across
                    # partitions via a 1x64 ones outer-product on the PE
                    rec = recp.tile([65, QT], F32R, tag="rec")
                    with nc.allow_low_precision(
                        reason="f32r-tagged recip output feeds the f32r broadcast matmul"
                    ):
                        nc.vector.reciprocal(rec[64:65, :], att_ps[HD : HD + 1, :])
                    rec_ps = ps_warm.tile([64, QT], F32, tag="warm")
                    nc.tensor.matmul(
                        rec_ps[:],
                        ones_sb[64:65, 0:64],
                        rec[64:65, :],
                        start=True,
                        stop=True,
                    )
                    rec_bc = recp.tile([64, QT], F32, tag="recbc")
                    nc.vector.tensor_copy(rec_bc[:], rec_ps[:])
                    nc.vector.tensor_tensor(
                        attT_sb[:, h, qsl], att_ps[0:HD, :], rec_bc[:], MULT
                    )

                # ---- partial O-projection for this q-tile ----
                for dt in range(4):
                    ps = ps_aux.tile([128, QT], F32, tag="aux")
                    for hc in range(HPG):
                        nc.tensor.matmul(
                            ps[:],
                            woT_sb[:, hc, 128 * dt : 128 * (dt + 1)],
                            attT_sb[:, hc, qsl],
                            start=(hc == 0),
                            stop=(hc == HPG - 1),
                        )
                    ot = outp.tile([128, QT], F32, tag="ot")
                    nc.vector.tensor_copy(ot[:], ps[:])
                    nc.sync.dma_start(outT[128 * dt : 128 * (dt + 1), qsl], ot[:])
    _split_excess_waits(nc)
    return nc


_CACHED_NC = None


def _get_nc():
    global _CACHED_NC
    if _CACHED_NC is None:
        _CACHED_NC = build_program()
    return _CACHED_NC


def _shard_inputs(query, Wk, Wv, Wo):
    wki = Wk.astype(np.float32) + np.eye(D, dtype=np.float32)
    in_maps = []
    perms = []
    for g in range(HG):
        perm = np.r_[DG * g : DG * (g + 1), 0 : DG * g, DG * (g + 1) : D]
        perms.append(perm)
    for n in range(N):
        for g in range(HG):
            perm = perms[g]
            hg = slice(DG * g, DG * (g + 1))
            qTn = np.ascontiguousarray(query[n].T[perm])          # [512, 2048]
            wkiT = np.ascontiguousarray(wki[hg, :][:, perm].T)    # [512, 256]
            wvT = np.ascontiguousarray(Wv[hg, :][:, perm].T)      # [512, 256]
            woT = np.ascontiguousarray(Wo[:, hg].T)               # [256, 512]
            in_maps.append(
                {
                    "qT": qTn.astype(np.float32),
                    "wkiT": wkiT.astype(np.float32),
                    "wvT": wvT.astype(np.float32),
                    "woT": woT.astype(np.float32),
                    "ones": np.ones((128, 512), dtype=np.float32),
                }
            )
    return in_maps


def run(query, Wk, Wv, Wo, **run_kwargs):
    """Run the SPMD kernel; returns (output, BassKernelResults)."""
    nc = _get_nc()
    in_maps = _shard_inputs(
        np.asarray(query, dtype=np.float32),
        np.asarray(Wk, dtype=np.float32),
        np.asarray(Wv, dtype=np.float32),
        np.asarray(Wo, dtype=np.float32),
    )
    res = run_bass_kernel_spmd(nc, in_maps, list(range(N * HG)), **run_kwargs)
    outs = []
    for n in range(N):
        pT = res.results[2 * n]["outT"] + res.results[2 * n + 1]["outT"]
        outs.append(pT.T)
    return np.stack(outs).astype(np.float32), res


def kernel(query, Wk, Wv, Wo):
    out, _ = run(query, Wk, Wv, Wo)
    return out


# revision 16
# speedup vs baseline: 1.3731x; 1.3731x over previous
"""Trainium2 Bass kernel for nn_MultiHeadAttention_39324720562623.

Reference computation (N=4, T=2048, D=512, H=8, HD=64), fp32:
    keys   = query @ Wk.T + query
    values = query @ Wv.T
    per head h: scores = softmax((Q_h @ K_h.T) / sqrt(HD))
                out_h  = scores @ V_h
    out = concat_heads(out_h) @ Wo.T

Sharding: 8 cores = 4 batches x 2 head-groups (4 heads each), pure SPMD.
Each core computes K/V projections for its head-group only, attention for
its 4 heads, and a partial O-projection; the host sums the two partials
per batch.  All operands are pre-transposed on the host so the kernel
works in "feature-major" (transposed) layouts throughout:

  qT   [512,2048]  query[n].T with feature dims permuted so this core's
                   head-group dims come first
  wkiT [512, 256]  ((Wk + I)[hg,:][:,perm]).T  -> K.T = wkiT.T @ qT
                   (residual folded into the weight)
  wvT  [512, 256]  (Wv[hg,:][:,perm]).T        -> V = qT_chunk.T @ wvT
  woT  [256, 512]  as [64,4,512]-striped: Wo.T rows for this head-group
  outT [512,2048]  partial out.T (host sums pairs, then transposes)

Softmax needs no max-subtraction: scores/8 are bounded (|s|<~16) so fp32
exp is safe.  The denominator comes for free from a ones-column appended
to V (row 64 of the att.T accumulation); normalization broadcasts the
reciprocal across partitions with a 1x64 ones outer-product on the PE.

All matmuls run as float32r (full PE rate at free-dim >= 256).
"""

import os
import sys

for _p in ("/opt/trn_rl_repo", os.path.join(os.path.expanduser("~"), ".axon_site", "_ro", "trn_rl_repo")):
    if os.path.isdir(_p) and _p not in sys.path:
        sys.path.insert(0, _p)
        break

import numpy as np

import concourse.bass as bass
import concourse.tile as tile
from concourse import mybir
from concourse.bass_utils import run_bass_kernel_spmd
from concourse.vector_clock import ScopedClock

N, T, D, H = 4, 2048, 512, 8
HD = D // H            # 64 head dim
HG = 2                 # head groups (cores per batch)
HPG = H // HG          # 4 heads per group
DG = D // HG           # 256 feature dims per group
F32 = mybir.dt.float32
F32R = mybir.dt.float32r
EXP = mybir.ActivationFunctionType.Exp
MULT = mybir.AluOpType.mult

QT = 512               # q-tile (matmul moving free dim)
NQT = T // QT          # 4
KC = 128               # k-chunk (partition dim of scores.T tiles)
NKC = T // KC          # 16
SGRP = 2               # k-chunks per scores-psum tile / exp instruction
NJUNK = 1              # PE-warmth filler matmuls per group (keeps HAM at K=8/8)
SCW = SGRP * QT        # 1024 scores tile free width


def _patch_drain():
    """walrus in this toolchain rejects >1 sync-wait on the kernel-tail
    Drain; split the waits across a chain of drains (1 wait each)."""
    if getattr(tile.TileContext, "_drain_split_patch", False):
        return

    def _drain_and_barrier(self, tick_clock, wait_clock):
        nc = self.nc
        d = nc.sync.drain()
        wait_clock.add_sem_waits(d.ins, ScopedClock({None: tick_clock.global_clock}))
        si = d.ins.sync_info
        waits = list(si.on_wait) if (si is not None and si.on_wait) else []
        if len(waits) > 1:
            si.on_wait = waits[:1]
            for w in waits[1:]:
                d2 = nc.sync.drain()
                if d2.ins.sync_info is None:
                    d2.ins.sync_info = mybir.SyncInfo(on_wait=[w], on_update=[])
                else:
                    d2.ins.sync_info.on_wait = [w]
        nc.all_engine_barrier()
        popped = nc._tile_sem_poison_stack.pop()
        assert popped is self._sem_poison
        nc.clear_and_free_semaphores(list(self.sems.allocated().values()))
        nc.all_engine_barrier()

    tile.TileContext._drain_and_barrier = _drain_and_barrier
    tile.TileContext._drain_split_patch = True


MAX_WAITS = 1


def _split_excess_waits(nc, maxw=MAX_WAITS):
    """walrus codegen rejects instructions with more than ~2 sync waits.
    Hoist excess waits onto same-engine nops inserted immediately before
    the offending instruction (same engine-stream position => identical
    semantics)."""
    nid = [0]

    def mk_nop(engine, waits):
        nid[0] += 1
        nop = mybir.InstNoOp(name=f"I-waitsplit-{nid[0]}")
        nop.engine = engine
        nop.sync_info = mybir.SyncInfo(on_wait=list(waits), on_update=[])
        try:
            nop.bass_nofuse = True
        except Exception:
            pass
        return nop

    for f in nc.m.functions:
        for bb in f.blocks:
            insts = bb.instructions
            i = 0
            while i < len(insts):
                ins = insts[i]
                si = ins.sync_info
                waits = list(si.on_wait) if (si is not None and si.on_wait) else []
                if len(waits) > maxw:
                    si.on_wait = waits[-maxw:]
                    excess = waits[:-maxw]
                    pos = i
                    for j in range(0, len(excess), maxw):
                        insts.insert(pos, mk_nop(ins.engine, excess[j : j + maxw]))
                        pos += 1
                        i += 1
                i += 1


def build_program():
    _patch_drain()
    nc = bass.Bass()
    qT = nc.dram_tensor("qT", [D, T], F32R, kind="ExternalInput")
    wkiT = nc.dram_tensor("wkiT", [D, DG], F32R, kind="ExternalInput")
    wvT = nc.dram_tensor("wvT", [D, DG], F32R, kind="ExternalInput")
    woT = nc.dram_tensor("woT", [DG, D], F32R, kind="ExternalInput")
    ones = nc.dram_tensor("ones", [128, 512], F32R, kind="ExternalInput")
    zeros = nc.dram_tensor("zeros", [64, T], F32R, kind="ExternalInput")
    outT = nc.dram_tensor("outT", [D, T], F32, kind="ExternalOutput")

    with tile.TileContext(nc) as tc:
        with (
            tc.tile_pool(name="singles", bufs=1) as singles,
            tc.tile_pool(name="pt", bufs=3) as ptp,
            tc.tile_pool(name="recp", bufs=3) as recp,
            tc.tile_pool(name="outp", bufs=3) as outp,
            tc.tile_pool(name="ps_sc", bufs=2, space="PSUM") as ps_sc,
            tc.tile_pool(name="ps_att", bufs=1, space="PSUM") as ps_att,
            tc.tile_pool(name="ps_warm", bufs=1, space="PSUM") as ps_warm,
            tc.tile_pool(name="ps_aux", bufs=2, space="PSUM") as ps_aux,
        ):
            qT_sb = singles.tile([128, 4, T], F32R)
            wkiT_sb = singles.tile([128, 4, DG], F32R)
            wvT_sb = singles.tile([128, 4, DG], F32R)
            woT_sb = singles.tile([64, HPG, D], F32R)
            kT_pad = singles.tile([128, HPG, T], F32R)
            # V augmented with a ones column per head: [128, kchunk, head, 65]
            vaug_sb = singles.tile([128, NKC, HPG, HD + 1], F32R)
            attT_sb = singles.tile([64, HPG, T], F32R)
            ones_sb = singles.tile([65, 512], F32R)

            # ---- input DMAs (chunked so compute can start early) ----
            for c in range(4):
                nc.sync.dma_start(
                    qT_sb[:, c], qT.rearrange("(c p) t -> c p t", p=128)[c]
                )
            nc.sync.dma_start(wkiT_sb[:], wkiT.rearrange("(c p) d -> p c d", p=128))
            nc.sync.dma_start(wvT_sb[:], wvT.rearrange("(c p) d -> p c d", p=128))
            nc.sync.dma_start(woT_sb[:], woT.rearrange("(c p) d -> p c d", p=64))
            nc.sync.dma_start(ones_sb[:], ones[0:65, :])
            nc.sync.dma_start(
                vaug_sb[:, :, :, HD],
                ones[:, 0:64].rearrange("p (a b) -> p a b", b=HPG),
            )
            for h in range(HPG):
                off = 64 - (h % 2) * 64  # complement of the head's parity slot
                nc.sync.dma_start(kT_pad[off : off + 64, h], zeros[:])

            # ---- K.T = (Wk+I) @ q.T for this head-group: [256, 2048] ----
            for dc in range(2):
                for tt in range(NQT):
                    ps = ps_aux.tile([128, QT], F32, tag="aux")
                    for di in range(4):
                        nc.tensor.matmul(
                            ps[:],
                            wkiT_sb[:, di, 128 * dc : 128 * (dc + 1)],
                            qT_sb[:, di, QT * tt : QT * (tt + 1)],
                            start=(di == 0),
                            stop=(di == 3),
                        )
                    tsl = slice(QT * tt, QT * (tt + 1))
                    nc.vector.tensor_copy(kT_pad[0:64, 2 * dc, tsl], ps[0:64, :])
                    nc.vector.tensor_copy(
                        kT_pad[64:128, 2 * dc + 1, tsl], ps[64:128, :]
                    )

            # ---- V = q @ Wv.T for this head-group: [2048, 256] ----
            for tci in range(NKC):
                ps = ps_aux.tile([128, DG], F32, tag="aux")
                for di in range(4):
                    nc.tensor.matmul(
                        ps[:],
                        qT_sb[:, di, 128 * tci : 128 * (tci + 1)],
                        wvT_sb[:, di],
                        start=(di == 0),
                        stop=(di == 3),
                    )
                nc.vector.tensor_copy(
                    vaug_sb[:, tci, :, 0:HD],
                    ps.rearrange("p (h d) -> p h d", d=HD),
                )

            # ---- attention per (q-tile, head) ----
            for jq in range(NQT):
                qsl = slice(QT * jq, QT * (jq + 1))
                for h in range(HPG):
                    ch = h // 2
                    att_ps = ps_att.tile([HD + 1, QT], F32, tag="att")
                    nmm = 0
                    for g0 in range(0, NKC, SGRP):
                        sc_ps = ps_sc.tile([128, SCW], F32, tag="sc")
                        for j in range(SGRP):
                            ik = g0 + j
                            nc.tensor.matmul(
                                sc_ps[:, QT * j : QT * (j + 1)],
                                kT_pad[:, h, 128 * ik : 128 * (ik + 1)],
                                qT_sb[:, ch, qsl],
                                start=True,
                                stop=True,
                            )
                        pt = ptp.tile([128, SCW], F32R, tag="pt")
                        nc.scalar.activation(pt[:], sc_ps[:], EXP, scale=0.125)
                        for j in range(SGRP):
                            ik = g0 + j
                            nc.tensor.matmul(
                                att_ps[:],
                                vaug_sb[:, ik, h],
                                pt[:, QT * j : QT * (j + 1)],
                                start=(nmm == 0),
                                stop=(nmm == NKC - 1),
                            )
                            nmm += 1
                        for _ in range(NJUNK):
                            wps = ps_warm.tile([64, QT], F32, tag="warm")
                            nc.tensor.matmul(
                                wps[0:1, :],
                                qT_sb[:, 0, 0:1],
                                qT_sb[:, 0, 0:QT],
                                start=True,
                                stop=True,
                            )
                    # normalization: recip of the ones-row, broadcast across
                    # partitions via a 1x64 ones outer-product on the PE
                    rec = recp.tile([65, QT], F32R, tag="rec")
                    with nc.allow_low_precision(
                        reason="f32r-tagged recip output feeds the f32r broadcast matmul"
                    ):
                        nc.vector.reciprocal(rec[64:65, :], att_ps[HD : HD + 1, :])
                    rec_ps = ps_aux.tile([64, QT], F32, tag="aux")
                    nc.tensor.matmul(
                        rec_ps[:],
                        ones_sb[64:65, 0:64],
                        rec[64:65, :],
                        start=True,
                        stop=True,
                    )
                    rec_bc = recp.tile([64, QT], F32, tag="recbc")
                    nc.vector.tensor_copy(rec_bc[:], rec_ps[:])
                    nc.vector.tensor_tensor(
                        attT_sb[:, h, qsl], att_ps[0:HD, :], rec_bc[:], MULT
                    )

                # ---- partial O-projection for this q-tile ----
                for dt in range(4):
                    ps = ps_aux.tile([128, QT], F32, tag="aux")
                    for hc in range(HPG):
                        nc.tensor.matmul(
                            ps[:],
                            woT_sb[:, hc, 128 * dt : 128 * (dt + 1)],
                            attT_sb[:, hc, qsl],
                            start=(hc == 0),
                            stop=(hc == HPG - 1),
                        )
                    ot = outp.tile([128, QT], F32, tag="ot")
                    nc.vector.tensor_copy(ot[:], ps[:])
                    nc.sync.dma_start(outT[128 * dt : 128 * (dt + 1), qsl], ot[:])
    _split_excess_waits(nc)
    return nc


_CACHED_NC = None


def _get_nc():
    global _CACHED_NC
    if _CACHED_NC is None:
        _CACHED_NC = build_program()
    return _CACHED_NC


def _shard_inputs(query, Wk, Wv, Wo):
    wki = Wk.astype(np.float32) + np.eye(D, dtype=np.float32)
    in_maps = []
    perms = []
    for g in range(HG):
        perm = np.r_[DG * g : DG * (g + 1), 0 : DG * g, DG * (g + 1) : D]
        perms.append(perm)
    for n in range(N):
        for g in range(HG):
            perm = perms[g]
            hg = slice(DG * g, DG * (g + 1))
            qTn = np.ascontiguousarray(query[n].T[perm])          # [512, 2048]
            wkiT = np.ascontiguousarray(wki[hg, :][:, perm].T)    # [512, 256]
            wvT = np.ascontiguousarray(Wv[hg, :][:, perm].T)      # [512, 256]
            woT = np.ascontiguousarray(Wo[:, hg].T)               # [256, 512]
            in_maps.append(
                {
                    "qT": qTn.astype(np.float32),
                    "wkiT": wkiT.astype(np.float32),
                    "wvT": wvT.astype(np.float32),
                    "woT": woT.astype(np.float32),
                    "ones": np.ones((128, 512), dtype=np.float32),
                    "zeros": np.zeros((64, T), dtype=np.float32),
                }
            )
    return in_maps


def run(query, Wk, Wv, Wo, **run_kwargs):
    """Run the SPMD kernel; returns (output, BassKernelResults)."""
    nc = _get_nc()
    in_maps = _shard_inputs(
        np.asarray(query, dtype=np.float32),
        np.asarray(Wk, dtype=np.float32),
        np.asarray(Wv, dtype=np.float32),
        np.asarray(Wo, dtype=np.float32),
    )
    res = run_bass_kernel_spmd(nc, in_maps, list(range(N * HG)), **run_kwargs)
    outs = []
    for n in range(N):
        pT = res.results[2 * n]["outT"] + res.results[2 * n + 1]["outT"]
        outs.append(pT.T)
    return np.stack(outs).astype(np.float32), res


def kernel(query, Wk, Wv, Wo):
    out, _ = run(query, Wk, Wv, Wo)
    return out


# revision 17
# speedup vs baseline: 1.4943x; 1.0882x over previous
"""Trainium2 Bass kernel for nn_MultiHeadAttention_39324720562623.

Reference computation (N=4, T=2048, D=512, H=8, HD=64), fp32:
    keys   = query @ Wk.T + query
    values = query @ Wv.T
    per head h: scores = softmax((Q_h @ K_h.T) / sqrt(HD))
                out_h  = scores @ V_h
    out = concat_heads(out_h) @ Wo.T

Sharding: 8 cores = 4 batches x 2 head-groups (4 heads each), pure SPMD.
Each core computes K/V projections for its head-group only, attention for
its 4 heads, and a partial O-projection; the host sums the two partials
per batch.  All operands are pre-transposed on the host so the kernel
works in "feature-major" (transposed) layouts throughout:

  qT   [512,2048]  query[n].T with feature dims permuted so this core's
                   head-group dims come first
  wkiT [512, 256]  ((Wk + I)[hg,:][:,perm]).T  -> K.T = wkiT.T @ qT
                   (residual folded into the weight)
  wvT  [512, 256]  (Wv[hg,:][:,perm]).T        -> V = qT_chunk.T @ wvT
  woT  [256, 512]  as [64,4,512]-striped: Wo.T rows for this head-group
  outT [512,2048]  partial out.T (host sums pairs, then transposes)

Softmax needs no max-subtraction: scores/8 are bounded (|s|<~16) so fp32
exp is safe.  The denominator comes for free from a ones-column appended
to V (row 64 of the att.T accumulation); normalization broadcasts the
reciprocal across partitions with a 1x64 ones outer-product on the PE.

All matmuls run as float32r (full PE rate at free-dim >= 256).
"""

import os
import sys

for _p in ("/opt/trn_rl_repo", os.path.join(os.path.expanduser("~"), ".axon_site", "_ro", "trn_rl_repo")):
    if os.path.isdir(_p) and _p not in sys.path:
        sys.path.insert(0, _p)
        break

import numpy as np

import concourse.bass as bass
import concourse.tile as tile
from concourse import mybir
from concourse.bass_utils import run_bass_kernel_spmd
from concourse.vector_clock import ScopedClock

N, T, D, H = 4, 2048, 512, 8
HD = D // H            # 64 head dim
HG = 2                 # head groups (cores per batch)
HPG = H // HG          # 4 heads per group
DG = D // HG           # 256 feature dims per group
F32 = mybir.dt.float32
F32R = mybir.dt.float32r
EXP = mybir.ActivationFunctionType.Exp
MULT = mybir.AluOpType.mult

QT = 512               # q-tile (matmul moving free dim)
NQT = T // QT          # 4
KC = 128               # k-chunk (partition dim of scores.T tiles)
NKC = T // KC          # 16
SGRP = 2               # k-chunks per scores-psum tile / exp instruction
NJUNK = 1              # PE-warmth filler matmuls per group (keeps HAM at K=8/8)
SCW = SGRP * QT        # 1024 scores tile free width


def _patch_drain():
    """walrus in this toolchain rejects >1 sync-wait on the kernel-tail
    Drain; split the waits across a chain of drains (1 wait each)."""
    if getattr(tile.TileContext, "_drain_split_patch", False):
        return

    def _drain_and_barrier(self, tick_clock, wait_clock):
        nc = self.nc
        d = nc.sync.drain()
        wait_clock.add_sem_waits(d.ins, ScopedClock({None: tick_clock.global_clock}))
        si = d.ins.sync_info
        waits = list(si.on_wait) if (si is not None and si.on_wait) else []
        if len(waits) > 1:
            si.on_wait = waits[:1]
            for w in waits[1:]:
                d2 = nc.sync.drain()
                if d2.ins.sync_info is None:
                    d2.ins.sync_info = mybir.SyncInfo(on_wait=[w], on_update=[])
                else:
                    d2.ins.sync_info.on_wait = [w]
        nc.all_engine_barrier()
        popped = nc._tile_sem_poison_stack.pop()
        assert popped is self._sem_poison
        nc.clear_and_free_semaphores(list(self.sems.allocated().values()))
        nc.all_engine_barrier()

    tile.TileContext._drain_and_barrier = _drain_and_barrier
    tile.TileContext._drain_split_patch = True


MAX_WAITS = 1


def _split_excess_waits(nc, maxw=MAX_WAITS):
    """walrus codegen rejects instructions with more than ~2 sync waits.
    Hoist excess waits onto same-engine nops inserted immediately before
    the offending instruction (same engine-stream position => identical
    semantics)."""
    nid = [0]

    def mk_nop(engine, waits):
        nid[0] += 1
        nop = mybir.InstNoOp(name=f"I-waitsplit-{nid[0]}")
        nop.engine = engine
        nop.sync_info = mybir.SyncInfo(on_wait=list(waits), on_update=[])
        try:
            nop.bass_nofuse = True
        except Exception:
            pass
        return nop

    for f in nc.m.functions:
        for bb in f.blocks:
            insts = bb.instructions
            i = 0
            while i < len(insts):
                ins = insts[i]
                si = ins.sync_info
                waits = list(si.on_wait) if (si is not None and si.on_wait) else []
                if len(waits) > maxw:
                    si.on_wait = waits[-maxw:]
                    excess = waits[:-maxw]
                    pos = i
                    for j in range(0, len(excess), maxw):
                        insts.insert(pos, mk_nop(ins.engine, excess[j : j + maxw]))
                        pos += 1
                        i += 1
                i += 1


def build_program():
    _patch_drain()
    nc = bass.Bass()
    qT = nc.dram_tensor("qT", [D, T], F32R, kind="ExternalInput")
    wkiT = nc.dram_tensor("wkiT", [D, DG], F32R, kind="ExternalInput")
    wvT = nc.dram_tensor("wvT", [D, DG], F32R, kind="ExternalInput")
    woT = nc.dram_tensor("woT", [HPG * 128, D], F32R, kind="ExternalInput")
    ones = nc.dram_tensor("ones", [128, 512], F32R, kind="ExternalInput")
    zeros = nc.dram_tensor("zeros", [64, T], F32R, kind="ExternalInput")
    sel = nc.dram_tensor("sel", [128, 64], F32R, kind="ExternalInput")
    outT = nc.dram_tensor("outT", [D, T], F32, kind="ExternalOutput")

    with tile.TileContext(nc) as tc:
        with (
            tc.tile_pool(name="singles", bufs=1) as singles,
            tc.tile_pool(name="pt", bufs=3) as ptp,
            tc.tile_pool(name="recp", bufs=3) as recp,
            tc.tile_pool(name="outp", bufs=3) as outp,
            tc.tile_pool(name="ps_sc", bufs=2, space="PSUM") as ps_sc,
            tc.tile_pool(name="ps_att", bufs=1, space="PSUM") as ps_att,
            tc.tile_pool(name="ps_warm", bufs=1, space="PSUM") as ps_warm,
            tc.tile_pool(name="ps_aux", bufs=2, space="PSUM") as ps_aux,
        ):
            qT_sb = singles.tile([128, 4, T], F32R)
            wkiT_sb = singles.tile([128, 4, DG], F32R)
            wvT_sb = singles.tile([128, 4, DG], F32R)
            woT_sb = singles.tile([128, HPG, D], F32R)
            sel_sb = singles.tile([128, 64], F32R)
            rec_t = singles.tile([128, QT], F32R)
            kT_pad = singles.tile([128, HPG, T], F32R)
            # V augmented with a ones column per head: [128, kchunk, head, 65]
            vaug_sb = singles.tile([128, NKC, HPG, HD + 1], F32R)
            attT_sb = singles.tile([128, HPG, T], F32R)
            ones_sb = singles.tile([65, 512], F32R)

            # ---- input DMAs (chunked so compute can start early) ----
            for c in range(4):
                nc.sync.dma_start(
                    qT_sb[:, c], qT.rearrange("(c p) t -> c p t", p=128)[c]
                )
            nc.sync.dma_start(wkiT_sb[:], wkiT.rearrange("(c p) d -> p c d", p=128))
            nc.sync.dma_start(wvT_sb[:], wvT.rearrange("(c p) d -> p c d", p=128))
            nc.sync.dma_start(woT_sb[:], woT.rearrange("(c p) d -> p c d", p=128))
            nc.sync.dma_start(sel_sb[:], sel[:])
            nc.sync.dma_start(rec_t[0:64, :], zeros[:, 0:QT])
            nc.sync.dma_start(rec_t[64:128, :], zeros[:, 0:QT])
            nc.sync.dma_start(ones_sb[:], ones[0:65, :])
            nc.sync.dma_start(
                vaug_sb[:, :, :, HD],
                ones[:, 0:64].rearrange("p (a b) -> p a b", b=HPG),
            )
            for h in range(HPG):
                off = 64 - (h % 2) * 64  # complement of the head's parity slot
                nc.sync.dma_start(kT_pad[off : off + 64, h], zeros[:])
                nc.sync.dma_start(attT_sb[64:128, h], zeros[:])

            # ---- K.T = (Wk+I) @ q.T for this head-group: [256, 2048] ----
            for dc in range(2):
                for tt in range(NQT):
                    ps = ps_aux.tile([128, QT], F32, tag="aux")
                    for di in range(4):
                        nc.tensor.matmul(
                            ps[:],
                            wkiT_sb[:, di, 128 * dc : 128 * (dc + 1)],
                            qT_sb[:, di, QT * tt : QT * (tt + 1)],
                            start=(di == 0),
                            stop=(di == 3),
                        )
                    tsl = slice(QT * tt, QT * (tt + 1))
                    nc.vector.tensor_copy(kT_pad[0:64, 2 * dc, tsl], ps[0:64, :])
                    nc.vector.tensor_copy(
                        kT_pad[64:128, 2 * dc + 1, tsl], ps[64:128, :]
                    )

            # ---- V = q @ Wv.T for this head-group: [2048, 256] ----
            for tci in range(NKC):
                ps = ps_aux.tile([128, DG], F32, tag="aux")
                for di in range(4):
                    nc.tensor.matmul(
                        ps[:],
                        qT_sb[:, di, 128 * tci : 128 * (tci + 1)],
                        wvT_sb[:, di],
                        start=(di == 0),
                        stop=(di == 3),
                    )
                nc.vector.tensor_copy(
                    vaug_sb[:, tci, :, 0:HD],
                    ps.rearrange("p (h d) -> p h d", d=HD),
                )

            # ---- attention per (q-tile, head) ----
            for jq in range(NQT):
                qsl = slice(QT * jq, QT * (jq + 1))
                for h in range(HPG):
                    ch = h // 2
                    att_ps = ps_att.tile([HD + 1, QT], F32, tag="att")
                    nmm = 0
                    for g0 in range(0, NKC, SGRP):
                        sc_ps = ps_sc.tile([128, SCW], F32, tag="sc")
                        for j in range(SGRP):
                            ik = g0 + j
                            nc.tensor.matmul(
                                sc_ps[:, QT * j : QT * (j + 1)],
                                kT_pad[:, h, 128 * ik : 128 * (ik + 1)],
                                qT_sb[:, ch, qsl],
                                start=True,
                                stop=True,
                            )
                        pt = ptp.tile([128, SCW], F32R, tag="pt")
                        nc.scalar.activation(pt[:], sc_ps[:], EXP, scale=0.125)
                        for j in range(SGRP):
                            ik = g0 + j
                            nc.tensor.matmul(
                                att_ps[:],
                                vaug_sb[:, ik, h],
                                pt[:, QT * j : QT * (j + 1)],
                                start=(nmm == 0),
                                stop=(nmm == NKC - 1),
                            )
                            nmm += 1
                        for _ in range(NJUNK):
                            wps = ps_warm.tile([64, QT], F32, tag="warm")
                            nc.tensor.matmul(
                                wps[0:1, :],
                                qT_sb[:, 0, 0:1],
                                qT_sb[:, 0, 0:QT],
                                start=True,
                                stop=True,
                            )
                    # normalization: recip of the ones-row, broadcast across
                    # partitions via a 1x64 ones outer-product on the PE
                    with nc.allow_low_precision(
                        reason="f32r-tagged recip output feeds the f32r broadcast matmul"
                    ):
                        nc.vector.reciprocal(rec_t[64:65, :], att_ps[HD : HD + 1, :])
                    rec_ps = ps_aux.tile([64, QT], F32, tag="aux")
                    nc.tensor.matmul(
                        rec_ps[:],
                        sel_sb[:],
                        rec_t[:],
                        start=True,
                        stop=True,
                    )
                    rec_bc = recp.tile([64, QT], F32, tag="recbc")
                    nc.vector.tensor_copy(rec_bc[:], rec_ps[:])
                    nc.vector.tensor_tensor(
                        attT_sb[0:64, h, qsl], att_ps[0:HD, :], rec_bc[:], MULT
                    )

                # ---- partial O-projection for this q-tile ----
                for dt in range(4):
                    ps = ps_aux.tile([128, QT], F32, tag="aux")
                    for hc in range(HPG):
                        nc.tensor.matmul(
                            ps[:],
                            woT_sb[:, hc, 128 * dt : 128 * (dt + 1)],
                            attT_sb[:, hc, qsl],
                            start=(hc == 0),
                            stop=(hc == HPG - 1),
                        )
                    ot = outp.tile([128, QT], F32, tag="ot")
                    nc.vector.tensor_copy(ot[:], ps[:])
                    nc.sync.dma_start(outT[128 * dt : 128 * (dt + 1), qsl], ot[:])
    _split_excess_waits(nc)
    return nc


_CACHED_NC = None


def _get_nc():
    global _CACHED_NC
    if _CACHED_NC is None:
        _CACHED_NC = build_program()
    return _CACHED_NC


def _sel_mat():
    s = np.zeros((128, 64), dtype=np.float32)
    s[64, :] = 1.0
    return s


def _shard_inputs(query, Wk, Wv, Wo):
    wki = Wk.astype(np.float32) + np.eye(D, dtype=np.float32)
    in_maps = []
    perms = []
    for g in range(HG):
        perm = np.r_[DG * g : DG * (g + 1), 0 : DG * g, DG * (g + 1) : D]
        perms.append(perm)
    for n in range(N):
        for g in range(HG):
            perm = perms[g]
            hg = slice(DG * g, DG * (g + 1))
            qTn = np.ascontiguousarray(query[n].T[perm])          # [512, 2048]
            wkiT = np.ascontiguousarray(wki[hg, :][:, perm].T)    # [512, 256]
            wvT = np.ascontiguousarray(Wv[hg, :][:, perm].T)      # [512, 256]
            woTc = np.ascontiguousarray(Wo[:, hg].T)              # [256, 512]
            woT = np.zeros((HPG * 128, D), dtype=np.float32)
            for hc in range(HPG):
                woT[128 * hc : 128 * hc + 64] = woTc[64 * hc : 64 * hc + 64]
            in_maps.append(
                {
                    "qT": qTn.astype(np.float32),
                    "wkiT": wkiT.astype(np.float32),
                    "wvT": wvT.astype(np.float32),
                    "woT": woT.astype(np.float32),
                    "ones": np.ones((128, 512), dtype=np.float32),
                    "zeros": np.zeros((64, T), dtype=np.float32),
                    "sel": _sel_mat(),
                }
            )
    return in_maps


def run(query, Wk, Wv, Wo, **run_kwargs):
    """Run the SPMD kernel; returns (output, BassKernelResults)."""
    nc = _get_nc()
    in_maps = _shard_inputs(
        np.asarray(query, dtype=np.float32),
        np.asarray(Wk, dtype=np.float32),
        np.asarray(Wv, dtype=np.float32),
        np.asarray(Wo, dtype=np.float32),
    )
    res = run_bass_kernel_spmd(nc, in_maps, list(range(N * HG)), **run_kwargs)
    outs = []
    for n in range(N):
        pT = res.results[2 * n]["outT"] + res.results[2 * n + 1]["outT"]
        outs.append(pT.T)
    return np.stack(outs).astype(np.float32), res


def kernel(query, Wk, Wv, Wo):
    out, _ = run(query, Wk, Wv, Wo)
    return out
